# revision 52
# baseline (speedup 1.0000x reference)
"""Trainium2 Bass kernel for nn_EqStftPBC (STFT perturbation-based compensation).

Sharding: core c in 0..7 handles n2 in {5c-20 .. 5c-16} for ALL four (b, m)
signals; the host sums the 8 partial deltas (K-split with host-side reduce).

Device pipeline per core (single SPMD program, identical across cores):
  STFT (X0 and Xs = roll(X, 5c-20), base shift folded into per-core DFT
  weights) -> residual rolls r=1..4 as permutation matmuls -> C = X0*conj(R)
  with paired-plane DVE ops (RiN plane makes both combines ADDs) -> U = M (*) C
  with the prev-frame roll-add folded into shifted-rhs matmuls -> V = U*R
  (per-j waves, last wave reads PSUM directly) -> Vsum (G is j-independent:
  D = G @ sum_j V_j, single 6-matmul pass) -> overlap-add folded into PSUM
  (Gb half writes at +1 column) -> evict + edge rescale -> DMA out.

Other folds: P^(1/3) scaled into the input frames (delta is cubic in x),
1/cov into the G weights, bias applied on the host.
"""

import numpy as np
from ml_dtypes import bfloat16

import concourse.bass as bass
import concourse.bacc as bacc
import concourse.mybir as mybir
import concourse.tile as tile

F = 80
T = 51
TP = 52
HOP = 40
L = 2080
BM = 4            # (b, m) units, all on every core
NJ = 5            # n2 per core: n2 = 5*core - 20 + r
CD = BM * T       # 204: dense (bm, t) slot per (plane, j)
WD = NJ * CD      # 1020: one plane across all j
FP32 = mybir.dt.float32
BF16 = mybir.dt.bfloat16
CPY = mybir.ActivationFunctionType.Copy


def _ap(t_ap, off, dims):
    return bass.AP(tensor=t_ap.tensor, offset=t_ap.offset + off,
                   ap=[t_ap.ap[0]] + dims)


def build_program(debug=False):
    nc = bacc.Bacc("TRN2", target_bir_lowering=False, debug=debug)

    xf = nc.dram_tensor("xf", [F, 3 * CD], BF16, kind="ExternalInput")
    fw = nc.dram_tensor("fw", [F, 4 * F], BF16, kind="ExternalInput")
    pw = nc.dram_tensor("pw", [F, 4 * F], BF16, kind="ExternalInput")
    mw = nc.dram_tensor("mw", [F, NJ * 3 * F], BF16, kind="ExternalInput")
    gw = nc.dram_tensor("gw", [F, 6 * HOP], BF16, kind="ExternalInput")
    yv = nc.dram_tensor("yv", [HOP, 2 * BM * TP], BF16, kind="ExternalOutput")

    MUL = mybir.AluOpType.mult
    ADD = mybir.AluOpType.add

    with tile.TileContext(nc) as tc:
        with (
            tc.tile_pool(name="const", bufs=1) as cpool,
            tc.tile_pool(name="work", bufs=1) as wpool,
            tc.tile_pool(name="ps_s", bufs=1, space="PSUM") as ps_s,
            tc.tile_pool(name="ps_u", bufs=6, space="PSUM") as ps_u,
        ):
            # ---- input DMAs spread across queues; STFT inputs first ----
            xfs = wpool.tile([F, 3 * CD], BF16, tag="xfs")
            nc.sync.dma_start(xfs[:, 0:CD], xf[:, 0:CD])
            nc.gpsimd.dma_start(xfs[:, CD:2 * CD], xf[:, CD:2 * CD])
            nc.scalar.dma_start(xfs[:, 2 * CD:3 * CD], xf[:, 2 * CD:3 * CD])
            fws = cpool.tile([F, 4 * F], BF16, tag="fws")
            nc.scalar.dma_start(fws[:, 2 * F:4 * F], fw[:, 2 * F:4 * F])
            nc.scalar.dma_start(fws[:, 0:2 * F], fw[:, 0:2 * F])
            pws = cpool.tile([F, 4 * F], BF16, tag="pws")
            nc.gpsimd.dma_start(pws[:, :], pw[:, :])
            mws = cpool.tile([F, NJ * 3 * F], BF16, tag="mws")
            HM = NJ * 3 * F // 2
            nc.gpsimd.dma_start(mws[:, 0:HM], mw[:, 0:HM])
            nc.sync.dma_start(mws[:, HM:2 * HM], mw[:, HM:2 * HM])
            gws = cpool.tile([F, 6 * HOP], BF16, tag="gws")
            nc.gpsimd.dma_start(gws[:, :], gw[:, :])

            # ---- STFT (Xs first: slot0 gates the R matmuls) ----
            Xsp = ps_u.tile([F, 2 * CD], FP32, tag="Up")
            X0p = ps_u.tile([F, 2 * CD], FP32, tag="Up")
            nc.tensor.matmul(Xsp[:, :], fws[:, 2 * F:3 * F], xfs[:, CD:3 * CD],
                             start=True, stop=False)
            nc.tensor.matmul(Xsp[:, :], fws[:, 3 * F:4 * F], xfs[:, 0:2 * CD],
                             start=False, stop=True)
            nc.tensor.matmul(X0p[:, :], fws[:, 0:F], xfs[:, CD:3 * CD],
                             start=True, stop=False)
            nc.tensor.matmul(X0p[:, :], fws[:, F:2 * F], xfs[:, 0:2 * CD],
                             start=False, stop=True)

            # Rall: plane-major [Rr(5j) | Ri(5j) | RiN(5j)], slot j = roll(Xs, j)
            Rall = wpool.tile([F, 3 * WD], BF16, tag="Rall")
            nc.scalar.activation(_ap(Rall[:, :], 0, [[WD, 2], [1, CD]]),
                                 Xsp[:, :], CPY)

            # X0T: [X0r x5 | X0i x5] tiled across j slots (tiling emitted
            # later, after C group {0} which reads slot 0 directly)
            X0T = wpool.tile([F, 2 * WD], BF16, tag="X0T")
            nc.scalar.activation(_ap(X0T[:, :], 0, [[WD, 2], [1, CD]]),
                                 X0p[:, :], CPY)

            def x0_tile():
                nc.vector.tensor_copy(
                    _ap(X0T[:, :], CD, [[1, 4 * CD]]),
                    X0T[:, None, 0:CD].to_broadcast([F, 4, CD]))
                nc.gpsimd.tensor_copy(
                    _ap(X0T[:, :], WD + CD, [[1, 4 * CD]]),
                    X0T[:, None, WD:WD + CD].to_broadcast([F, 4, CD]))

            # ---- residual rolls r=1..4 via permutation matmuls ----
            def r_roll(r):
                Rp = ps_u.tile([F, 2 * CD], FP32, tag="Up")
                rhs = _ap(Rall[:, :], 0, [[WD, 2], [1, CD]])
                nc.tensor.matmul(Rp[:, :], pws[:, (r - 1) * F:r * F], rhs,
                                 start=True, stop=True)
                nc.scalar.activation(
                    _ap(Rall[:, :], r * CD, [[WD, 2], [1, CD]]),
                    _ap(Rp[:, :], 0, [[CD, 2], [1, CD]]), CPY)

            def ri_neg(j, eng='s'):
                dst = _ap(Rall[:, :], 2 * WD + j * CD, [[1, CD]])
                srcv = _ap(Rall[:, :], WD + j * CD, [[1, CD]])
                if eng == 'v':
                    nc.vector.tensor_scalar_mul(dst, srcv, -1.0)
                else:
                    nc.scalar.activation(dst, srcv, CPY, scale=-1.0)

            # ---- C stage (grouped, paired planes; roll-add folded into U) ----
            CS = wpool.tile([F, 4 * WD], BF16, tag="CS")  # [sA5|sC5|sB5|sDN5]
            Cp = wpool.tile([F, 2 * WD], BF16, tag="Cp")
            TTv = nc.vector.tensor_tensor
            TTg = nc.gpsimd.tensor_tensor

            def c_group(j0, nj, eng2):
                o = j0 * CD
                n = nj * CD
                # [sA|sC] = [X0r|X0i] (x) Rr ; [sB|sDN] = [X0i|X0r] (x) [Ri|RiN]
                TTv(_ap(CS[:, :], o, [[WD, 2], [1, n]]),
                    _ap(X0T[:, :], o, [[WD, 2], [1, n]]),
                    _ap(Rall[:, :], o, [[0, 2], [1, n]]), MUL)
                (TTv if eng2 == 'v' else TTg)(
                    _ap(CS[:, :], 2 * WD + o, [[WD, 2], [1, n]]),
                    _ap(X0T[:, :], WD + o, [[-WD, 2], [1, n]]),
                    _ap(Rall[:, :], WD + o, [[WD, 2], [1, n]]), MUL)
                # [Crp|Cip] = [sA|sC] + [sB|sDN]
                TTv(_ap(Cp[:, :], o, [[WD, 2], [1, n]]),
                    _ap(CS[:, :], o, [[WD, 2], [1, n]]),
                    _ap(CS[:, :], 2 * WD + o, [[WD, 2], [1, n]]), ADD)

            # ---- per-j stages ----
            VS = wpool.tile([F, 4 * WD], BF16, tag="VS")   # tA5|tBN5|tC5|tD5
            Vall = wpool.tile([F, 2 * WD], BF16, tag="Vall")  # Vr5 | Vi5
            VQ = wpool.tile([F, 8 * CD], BF16, tag="VQ")  # S01|S012|S0123|Pall
            Ue = wpool.tile([F, 2 * WD], BF16, tag="Ue")   # Ur5 | Ui5
            Y = ps_s.tile([HOP, 2 * BM * TP], FP32, tag="Y")
            nc.vector.memset(Y[:, :], 0)
            Ups = [None] * NJ

            def u_mm(j):
                Up = ps_u.tile([F, 2 * CD], FP32, tag="Up")
                Ups[j] = Up
                mo = j * 3 * F
                o = j * CD
                shv = [[WD, 2], [T, BM], [1, T - 1]]
                shd = [[CD, 2], [T, BM], [1, T - 1]]
                wrv = [[WD, 2], [T, BM]]
                wrd = [[CD, 2], [T, BM]]
                for k, (wo, ro, dlo, dn) in enumerate((
                        (mo, o, 0, 2 * CD),            # Mr  @ [Cr|Ci]
                        (mo + F, WD + o, 0, CD),       # MiN @ Ci -> Ur half
                        (mo + 2 * F, o, CD, CD))):     # Mi  @ Cr -> Ui half
                    w = mws[:, wo:wo + F]
                    if dn == 2 * CD:
                        rhs0 = _ap(Cp[:, :], ro, [[WD, 2], [1, CD]])
                        dst0 = Up[:, 0:2 * CD]
                        rhs1 = _ap(Cp[:, :], ro, shv)
                        dst1 = _ap(Up[:, :], 1, shd)
                        rhs2 = _ap(Cp[:, :], ro + T - 1, wrv)
                        dst2 = _ap(Up[:, :], 0, wrd)
                    else:
                        rhs0 = _ap(Cp[:, :], ro, [[1, CD]])
                        dst0 = Up[:, dlo:dlo + CD]
                        rhs1 = _ap(Cp[:, :], ro, [[T, BM], [1, T - 1]])
                        dst1 = _ap(Up[:, :], dlo + 1, [[T, BM], [1, T - 1]])
                        rhs2 = _ap(Cp[:, :], ro + T - 1, [[T, BM]])
                        dst2 = _ap(Up[:, :], dlo, [[T, BM]])
                    nc.tensor.matmul(dst0, w, rhs0, start=(k == 0), stop=False)
                    nc.tensor.matmul(dst1, w, rhs1, start=False, stop=False)
                    nc.tensor.matmul(dst2, w, rhs2, start=False,
                                     stop=(k == 2))

            def u_evict(j):
                # plane-major: Ur -> Ue[j*CD], Ui -> Ue[WD + j*CD]
                nc.scalar.activation(_ap(Ue[:, :], j * CD, [[WD, 2], [1, CD]]),
                                     Ups[j][:, :], CPY)

            def v_wave(j0, nj, psum=False):
                o = j0 * CD
                n = nj * CD
                uu = (Ups[j0][:, :] if psum
                      else _ap(Ue[:, :], o, [[WD, 2], [1, n]]))
                # [tA|tBN] = [Ur|Ui] (x) [Rr|RiN]; [tC|tD] = [Ur|Ui] (x) [Ri|Rr]
                TTv(_ap(VS[:, :], o, [[2 * WD, 2], [1, n]]), uu,
                    _ap(Rall[:, :], o, [[2 * WD, 2], [1, n]]), MUL)
                TTv(_ap(VS[:, :], WD + o, [[2 * WD, 2], [1, n]]), uu,
                    _ap(Rall[:, :], WD + o, [[-WD, 2], [1, n]]), MUL)
                # [Vr|Vi] = [tA|tC] + [tBN|tD]
                TTv(_ap(Vall[:, :], o, [[WD, 2], [1, n]]),
                    _ap(VS[:, :], o, [[WD, 2], [1, n]]),
                    _ap(VS[:, :], 2 * WD + o, [[WD, 2], [1, n]]), ADD)

            CW = BM * TP

            def d_pass(rhs2, rhs_i, rhs_r, stop):
                d2a = _ap(Y[:, :], 0, [[CW, 2], [TP, BM], [1, T]])
                d2b = _ap(Y[:, :], 1, [[CW, 2], [TP, BM], [1, T]])
                dia = _ap(Y[:, :], 0, [[TP, BM], [1, T]])
                dib = _ap(Y[:, :], 1, [[TP, BM], [1, T]])
                dra = _ap(Y[:, :], CW, [[TP, BM], [1, T]])
                drb = _ap(Y[:, :], CW + 1, [[TP, BM], [1, T]])
                nc.tensor.matmul(d2a, gws[:, 0:HOP], rhs2,
                                 start=False, stop=False)
                nc.tensor.matmul(d2b, gws[:, HOP:2 * HOP], rhs2,
                                 start=False, stop=False)
                nc.tensor.matmul(dia, gws[:, 2 * HOP:3 * HOP], rhs_i,
                                 start=False, stop=False)
                nc.tensor.matmul(dib, gws[:, 3 * HOP:4 * HOP], rhs_i,
                                 start=False, stop=False)
                nc.tensor.matmul(dra, gws[:, 4 * HOP:5 * HOP], rhs_r,
                                 start=False, stop=stop)
                nc.tensor.matmul(drb, gws[:, 5 * HOP:6 * HOP], rhs_r,
                                 start=False, stop=stop)

            # ---- software-pipelined emission ----
            ri_neg(0, 'v')
            r_roll(1)
            r_roll(2)
            r_roll(3)
            r_roll(4)
            c_group(0, 1, 'v')
            u_mm(0)
            u_evict(0)
            x0_tile()
            ri_neg(1, 'v')
            ri_neg(2, 'v')
            c_group(1, 2, 'v')
            u_mm(1)
            u_evict(1)
            ri_neg(3, 'v')
            ri_neg(4, 'v')
            c_group(3, 2, 'v')
            u_mm(2)
            u_evict(2)
            # V waves per j; incremental sums on gpsimd
            def vq_ap(k):
                return _ap(VQ[:, :], k * CD, [[4 * CD, 2], [1, CD]])

            def vall_ap(j):
                return _ap(Vall[:, :], j * CD, [[WD, 2], [1, CD]])

            v_wave(0, 1)
            u_mm(3)
            u_evict(3)
            v_wave(1, 1)
            TTv(vq_ap(0), vall_ap(0), vall_ap(1), ADD)
            u_mm(4)
            v_wave(2, 1)
            TTv(vq_ap(1), vq_ap(0), vall_ap(2), ADD)
            v_wave(3, 1)
            TTv(vq_ap(2), vq_ap(1), vall_ap(3), ADD)
            v_wave(4, 1, psum=True)
            TTv(vq_ap(3), vq_ap(2), vall_ap(4), ADD)
            d_pass(vq_ap(3),
                   _ap(VQ[:, :], 7 * CD, [[1, CD]]),
                   _ap(VQ[:, :], 3 * CD, [[1, CD]]), True)

            # ---- tail: evict Y (cov/2 folded in G), rescale edge cols ----
            Ysb = wpool.tile([HOP, 2 * BM * TP], BF16, tag="Ysb")
            nc.scalar.activation(Ysb[:, :], Y[:, :], CPY)
            nc.scalar.activation(
                _ap(Ysb[:, :], 0, [[CW, 2], [TP, BM], [T, 2]]),
                _ap(Y[:, :], 0, [[CW, 2], [TP, BM], [T, 2]]), CPY, scale=2.0)
            nc.sync.dma_start(yv[:, 0:CW], Ysb[:, 0:CW])
            nc.gpsimd.dma_start(yv[:, CW:2 * CW], Ysb[:, CW:2 * CW])
    return nc


# ---------------- host side ----------------

def _dft_consts():
    j = np.arange(F)
    W = np.exp(-2j * np.pi * np.outer(j, j) / F)
    G = np.exp(+2j * np.pi * np.outer(j, j) / F) / F
    return W, G


def _frame(sig):
    idx = np.arange(T)[None, :] * HOP + np.arange(F)[:, None]   # [g, t]
    return sig[idx].astype(np.float32)


def _m_mats(w2, n2):
    g = np.arange(F)[:, None]
    f = np.arange(F)[None, :]
    n1 = ((f - g + 20) % F) - 20
    valid = (n1 >= -20) & (n1 <= 19)
    n1c = np.clip(n1 + 20, 0, 39)
    col = w2[:, n2 + 20]
    Mr = np.where(valid, col.real[n1c], 0.0).astype(np.float32)
    Mi = np.where(valid, col.imag[n1c], 0.0).astype(np.float32)
    return Mr, Mi


def make_in_maps(x_real, x_imag, task_info, w_real, w_imag):
    W, G = _dft_consts()
    b, _, m = x_real.shape
    P = np.power(10.0, task_info[:, 0] / 10.0) / m
    w2 = (np.asarray(w_real) + 1j * np.asarray(w_imag)).reshape(40, 40)

    frs, fis = [], []
    for bb in range(b):
        s = float(P[bb]) ** (1.0 / 3.0)
        for mm in range(m):
            frs.append(_frame(x_real[bb, :, mm]) * s)
            fis.append(_frame(x_imag[bb, :, mm]) * s)
    fr = np.stack(frs, 1)
    fi = np.stack(fis, 1)
    xfv = np.concatenate([(-fi).reshape(F, -1), fr.reshape(F, -1),
                          fi.reshape(F, -1)], axis=1).astype(bfloat16)

    # G folded: 1/cov=1/2, rows split [0:40)/[40:80) for fused overlap-add
    Gh = G * 0.5
    gwv = np.concatenate([Gh.real[0:HOP].T, Gh.real[HOP:F].T,
                          -Gh.imag[0:HOP].T, -Gh.imag[HOP:F].T,
                          Gh.imag[0:HOP].T, Gh.imag[HOP:F].T],
                         axis=1).astype(bfloat16)

    # permutation matrices for rolls r=1..4 (lhsT[g, f] = 1 iff g=(f-r)%80)
    pparts = []
    g = np.arange(F)
    for r in range(1, NJ):
        Pm = np.zeros((F, F), np.float32)
        Pm[(g - r) % F, g] = 1.0
        pparts.append(Pm)
    pwv = np.concatenate(pparts, axis=1).astype(bfloat16)

    in_maps, shards = [], []
    for ci in range(8):
        sc = 5 * ci - 20
        Ws = np.roll(W, sc, axis=0).T
        fwv = np.concatenate([W.real, W.imag, Ws.real, Ws.imag],
                             axis=1).astype(bfloat16)
        mparts = []
        for r in range(NJ):
            Mr, Mi = _m_mats(w2, sc + r)
            mparts += [Mr, -Mi, Mi]
        mwv = np.concatenate(mparts, axis=1).astype(bfloat16)
        in_maps.append({"xf": xfv, "fw": fwv, "pw": pwv, "mw": mwv,
                        "gw": gwv})
        shards.append(ci)

    cov = np.zeros(L)
    idx = (np.arange(T)[:, None] * HOP + np.arange(F)[None, :]).reshape(-1)
    np.add.at(cov, idx, 1.0)
    cov = np.where(cov > 0, cov, 1.0)
    return in_maps, shards, P, cov


_NC_CACHE = {}


def kernel(x_real, x_imag, task_info, w_real, w_imag, b_real, b_imag):
    x_real = np.asarray(x_real)
    x_imag = np.asarray(x_imag)
    task_info = np.asarray(task_info)
    b, Lx, m = x_real.shape
    assert (b, Lx, m) == (2, L, 2)

    if "nc" not in _NC_CACHE:
        nc_ = build_program(debug=False)
        nc_.compile()
        _NC_CACHE["nc"] = nc_
    nc = _NC_CACHE["nc"]

    in_maps, shards, P, cov = make_in_maps(x_real, x_imag, task_info,
                                           w_real, w_imag)
    from concourse.bass_utils import run_bass_kernel_spmd
    res = run_bass_kernel_spmd(nc, in_maps, list(range(8))).results

    CW = BM * TP
    Ysum = np.zeros((HOP, 2 * CW), np.float64)
    for i in range(8):
        Ysum += np.asarray(res[i]["yv"], np.float64)
    Y = Ysum.reshape(HOP, 2, BM, TP)

    x = (x_real + 1j * x_imag).astype(np.complex64)
    out = x.copy()
    bias = complex(np.asarray(b_real)[0], np.asarray(b_imag)[0])
    bias_sig = np.zeros(L, np.complex64)
    bias_sig[np.arange(T) * HOP] = bias
    bias_sig /= cov
    for u in range(BM):
        bb, mm = divmod(u, m)
        yr = Y[:, 0, u].T.ravel()[:L]
        yi = Y[:, 1, u].T.ravel()[:L]
        out[bb, :, mm] += (yr + 1j * yi).astype(np.complex64)
        out[bb, :, mm] += (P[bb] * bias_sig).astype(np.complex64)
    return out[:, 20:L - 20, :]


# revision 53
# speedup vs baseline: 1.2655x; 1.2655x over previous
"""Trainium2 Bass kernel for nn_EqStftPBC (STFT perturbation-based compensation).

Sharding: core c in 0..7 handles n2 in {5c-20 .. 5c-16} for ALL four (b, m)
signals; the host sums the 8 partial deltas (K-split with host-side reduce).

Device pipeline per core (single SPMD program, identical across cores):
  STFT (X0 and Xs = roll(X, 5c-20), base shift folded into per-core DFT
  weights) -> residual rolls r=1..4 as permutation matmuls -> C = X0*conj(R)
  with paired-plane DVE ops (RiN plane makes both combines ADDs) -> U = M (*) C
  with the prev-frame roll-add folded into shifted-rhs matmuls -> V = U*R
  (per-j waves, last wave reads PSUM directly) -> Vsum (G is j-independent:
  D = G @ sum_j V_j, single 6-matmul pass) -> overlap-add folded into PSUM
  (Gb half writes at +1 column) -> evict + edge rescale -> DMA out.

Other folds: P^(1/3) scaled into the input frames (delta is cubic in x),
1/cov into the G weights, bias applied on the host.
"""

import numpy as np
from ml_dtypes import bfloat16

import concourse.bass as bass
import concourse.bacc as bacc
import concourse.mybir as mybir
import concourse.tile as tile

F = 80
T = 51
TP = 52
HOP = 40
L = 2080
BM = 4            # (b, m) units, all on every core
NJ = 5            # n2 per core: n2 = 5*core - 20 + r
CD = BM * T       # 204: dense (bm, t) slot per (plane, j)
WD = NJ * CD      # 1020: one plane across all j
FP32 = mybir.dt.float32
BF16 = mybir.dt.bfloat16
CPY = mybir.ActivationFunctionType.Copy


def _ap(t_ap, off, dims):
    return bass.AP(tensor=t_ap.tensor, offset=t_ap.offset + off,
                   ap=[t_ap.ap[0]] + dims)


def build_program(debug=False):
    nc = bacc.Bacc("TRN2", target_bir_lowering=False, debug=debug)

    xf = nc.dram_tensor("xf", [F, 3 * CD], BF16, kind="ExternalInput")
    fw = nc.dram_tensor("fw", [F, 4 * F], BF16, kind="ExternalInput")
    pw = nc.dram_tensor("pw", [F, 4 * F], BF16, kind="ExternalInput")
    mw = nc.dram_tensor("mw", [F, NJ * 3 * F], BF16, kind="ExternalInput")
    gw = nc.dram_tensor("gw", [F, 6 * HOP], BF16, kind="ExternalInput")
    yv = nc.dram_tensor("yv", [HOP, 2 * BM * TP], BF16, kind="ExternalOutput")

    MUL = mybir.AluOpType.mult
    ADD = mybir.AluOpType.add

    with tile.TileContext(nc) as tc:
        with (
            tc.tile_pool(name="const", bufs=1) as cpool,
            tc.tile_pool(name="work", bufs=1) as wpool,
            tc.tile_pool(name="ps_s", bufs=1, space="PSUM") as ps_s,
            tc.tile_pool(name="ps_u", bufs=6, space="PSUM") as ps_u,
        ):
            # ---- input DMAs spread across queues; STFT inputs first ----
            xfs = wpool.tile([F, 3 * CD], BF16, tag="xfs")
            HX = 3 * CD // 2
            nc.sync.dma_start(xfs[:, 0:HX], xf[:, 0:HX])
            nc.gpsimd.dma_start(xfs[:, HX:3 * CD], xf[:, HX:3 * CD])
            fws = cpool.tile([F, 4 * F], BF16, tag="fws")
            nc.scalar.dma_start(fws[:, 2 * F:4 * F], fw[:, 2 * F:4 * F])
            nc.scalar.dma_start(fws[:, 0:2 * F], fw[:, 0:2 * F])
            pws = cpool.tile([F, 4 * F], BF16, tag="pws")
            nc.gpsimd.dma_start(pws[:, :], pw[:, :])
            mws = cpool.tile([F, NJ * 3 * F], BF16, tag="mws")
            HM = NJ * 3 * F // 2
            nc.gpsimd.dma_start(mws[:, 0:HM], mw[:, 0:HM])
            nc.sync.dma_start(mws[:, HM:2 * HM], mw[:, HM:2 * HM])
            gws = cpool.tile([F, 6 * HOP], BF16, tag="gws")
            nc.gpsimd.dma_start(gws[:, :], gw[:, :])

            # ---- STFT (Xs first: slot0 gates the R matmuls) ----
            Xsp = ps_u.tile([F, 2 * CD], FP32, tag="Up")
            X0p = ps_u.tile([F, 2 * CD], FP32, tag="Up")
            nc.tensor.matmul(Xsp[:, :], fws[:, 2 * F:3 * F], xfs[:, CD:3 * CD],
                             start=True, stop=False)
            nc.tensor.matmul(Xsp[:, :], fws[:, 3 * F:4 * F], xfs[:, 0:2 * CD],
                             start=False, stop=True)
            nc.tensor.matmul(X0p[:, :], fws[:, 0:F], xfs[:, CD:3 * CD],
                             start=True, stop=False)
            nc.tensor.matmul(X0p[:, :], fws[:, F:2 * F], xfs[:, 0:2 * CD],
                             start=False, stop=True)

            # Rall: plane-major [Rr(5j) | Ri(5j) | RiN(5j)], slot j = roll(Xs, j)
            Rall = wpool.tile([F, 3 * WD], BF16, tag="Rall")
            nc.scalar.activation(_ap(Rall[:, :], 0, [[WD, 2], [1, CD]]),
                                 Xsp[:, :], CPY)

            # X0T: [X0r x5 | X0i x5] tiled across j slots (tiling emitted
            # later, after C group {0} which reads slot 0 directly)
            X0T = wpool.tile([F, 2 * WD], BF16, tag="X0T")
            nc.scalar.activation(_ap(X0T[:, :], 0, [[WD, 2], [1, CD]]),
                                 X0p[:, :], CPY)

            def x0_tile():
                for pl in range(2):
                    nc.vector.tensor_copy(
                        _ap(X0T[:, :], pl * WD + CD, [[1, 4 * CD]]),
                        X0T[:, None, pl * WD:pl * WD + CD].to_broadcast(
                            [F, 4, CD]))

            # ---- residual rolls r=1..4 via permutation matmuls ----
            def r_roll(r):
                Rp = ps_u.tile([F, 2 * CD], FP32, tag="Up")
                rhs = _ap(Rall[:, :], 0, [[WD, 2], [1, CD]])
                nc.tensor.matmul(Rp[:, :], pws[:, (r - 1) * F:r * F], rhs,
                                 start=True, stop=True)
                nc.scalar.activation(
                    _ap(Rall[:, :], r * CD, [[WD, 2], [1, CD]]),
                    _ap(Rp[:, :], 0, [[CD, 2], [1, CD]]), CPY)

            def ri_neg(j, eng='s'):
                dst = _ap(Rall[:, :], 2 * WD + j * CD, [[1, CD]])
                srcv = _ap(Rall[:, :], WD + j * CD, [[1, CD]])
                if eng == 'v':
                    nc.vector.tensor_scalar_mul(dst, srcv, -1.0)
                else:
                    nc.scalar.activation(dst, srcv, CPY, scale=-1.0)

            # ---- C stage (grouped, paired planes; roll-add folded into U) ----
            CS = wpool.tile([F, 4 * WD], BF16, tag="CS")  # [sA5|sC5|sB5|sDN5]
            Cp = wpool.tile([F, 2 * WD], BF16, tag="Cp")
            TTv = nc.vector.tensor_tensor
            TTg = nc.gpsimd.tensor_tensor

            def c_group(j0, nj, eng2):
                o = j0 * CD
                n = nj * CD
                # [sA|sC] = [X0r|X0i] (x) Rr ; [sB|sDN] = [X0i|X0r] (x) [Ri|RiN]
                TTv(_ap(CS[:, :], o, [[WD, 2], [1, n]]),
                    _ap(X0T[:, :], o, [[WD, 2], [1, n]]),
                    _ap(Rall[:, :], o, [[0, 2], [1, n]]), MUL)
                (TTv if eng2 == 'v' else TTg)(
                    _ap(CS[:, :], 2 * WD + o, [[WD, 2], [1, n]]),
                    _ap(X0T[:, :], WD + o, [[-WD, 2], [1, n]]),
                    _ap(Rall[:, :], WD + o, [[WD, 2], [1, n]]), MUL)
                # [Crp|Cip] = [sA|sC] + [sB|sDN]
                TTv(_ap(Cp[:, :], o, [[WD, 2], [1, n]]),
                    _ap(CS[:, :], o, [[WD, 2], [1, n]]),
                    _ap(CS[:, :], 2 * WD + o, [[WD, 2], [1, n]]), ADD)

            # ---- per-j stages ----
            VS = wpool.tile([F, 4 * WD], BF16, tag="VS")   # tA5|tBN5|tC5|tD5
            Vall = wpool.tile([F, 2 * WD], BF16, tag="Vall")  # Vr5 | Vi5
            VQ = wpool.tile([F, 8 * CD], BF16, tag="VQ")  # S01|S012|S0123|Pall
            Ue = wpool.tile([F, 2 * WD], BF16, tag="Ue")   # Ur5 | Ui5
            Y = ps_s.tile([HOP, 2 * BM * TP], FP32, tag="Y")
            nc.vector.memset(Y[:, :], 0)
            Ups = [None] * NJ

            def u_mm(j):
                Up = ps_u.tile([F, 2 * CD], FP32, tag="Up")
                Ups[j] = Up
                mo = j * 3 * F
                o = j * CD
                shv = [[WD, 2], [T, BM], [1, T - 1]]
                shd = [[CD, 2], [T, BM], [1, T - 1]]
                wrv = [[WD, 2], [T, BM]]
                wrd = [[CD, 2], [T, BM]]
                for k, (wo, ro, dlo, dn) in enumerate((
                        (mo, o, 0, 2 * CD),            # Mr  @ [Cr|Ci]
                        (mo + F, WD + o, 0, CD),       # MiN @ Ci -> Ur half
                        (mo + 2 * F, o, CD, CD))):     # Mi  @ Cr -> Ui half
                    w = mws[:, wo:wo + F]
                    if dn == 2 * CD:
                        rhs0 = _ap(Cp[:, :], ro, [[WD, 2], [1, CD]])
                        dst0 = Up[:, 0:2 * CD]
                        rhs1 = _ap(Cp[:, :], ro, shv)
                        dst1 = _ap(Up[:, :], 1, shd)
                        rhs2 = _ap(Cp[:, :], ro + T - 1, wrv)
                        dst2 = _ap(Up[:, :], 0, wrd)
                    else:
                        rhs0 = _ap(Cp[:, :], ro, [[1, CD]])
                        dst0 = Up[:, dlo:dlo + CD]
                        rhs1 = _ap(Cp[:, :], ro, [[T, BM], [1, T - 1]])
                        dst1 = _ap(Up[:, :], dlo + 1, [[T, BM], [1, T - 1]])
                        rhs2 = _ap(Cp[:, :], ro + T - 1, [[T, BM]])
                        dst2 = _ap(Up[:, :], dlo, [[T, BM]])
                    nc.tensor.matmul(dst0, w, rhs0, start=(k == 0), stop=False)
                    nc.tensor.matmul(dst1, w, rhs1, start=False, stop=False)
                    nc.tensor.matmul(dst2, w, rhs2, start=False,
                                     stop=(k == 2))

            def u_evict(j):
                # plane-major: Ur -> Ue[j*CD], Ui -> Ue[WD + j*CD]
                nc.scalar.activation(_ap(Ue[:, :], j * CD, [[WD, 2], [1, CD]]),
                                     Ups[j][:, :], CPY)

            def v_wave(j0, nj, psum=False):
                o = j0 * CD
                n = nj * CD
                uu = (Ups[j0][:, :] if psum
                      else _ap(Ue[:, :], o, [[WD, 2], [1, n]]))
                # [tA|tBN] = [Ur|Ui] (x) [Rr|RiN]; [tC|tD] = [Ur|Ui] (x) [Ri|Rr]
                TTv(_ap(VS[:, :], o, [[2 * WD, 2], [1, n]]), uu,
                    _ap(Rall[:, :], o, [[2 * WD, 2], [1, n]]), MUL)
                TTv(_ap(VS[:, :], WD + o, [[2 * WD, 2], [1, n]]), uu,
                    _ap(Rall[:, :], WD + o, [[-WD, 2], [1, n]]), MUL)
                # [Vr|Vi] = [tA|tC] + [tBN|tD]
                TTv(_ap(Vall[:, :], o, [[WD, 2], [1, n]]),
                    _ap(VS[:, :], o, [[WD, 2], [1, n]]),
                    _ap(VS[:, :], 2 * WD + o, [[WD, 2], [1, n]]), ADD)

            CW = BM * TP

            def d_pass(rhs2, rhs_i, rhs_r, stop):
                d2a = _ap(Y[:, :], 0, [[CW, 2], [TP, BM], [1, T]])
                d2b = _ap(Y[:, :], 1, [[CW, 2], [TP, BM], [1, T]])
                dia = _ap(Y[:, :], 0, [[TP, BM], [1, T]])
                dib = _ap(Y[:, :], 1, [[TP, BM], [1, T]])
                dra = _ap(Y[:, :], CW, [[TP, BM], [1, T]])
                drb = _ap(Y[:, :], CW + 1, [[TP, BM], [1, T]])
                nc.tensor.matmul(d2a, gws[:, 0:HOP], rhs2,
                                 start=False, stop=False)
                nc.tensor.matmul(d2b, gws[:, HOP:2 * HOP], rhs2,
                                 start=False, stop=False)
                nc.tensor.matmul(dia, gws[:, 2 * HOP:3 * HOP], rhs_i,
                                 start=False, stop=False)
                nc.tensor.matmul(dib, gws[:, 3 * HOP:4 * HOP], rhs_i,
                                 start=False, stop=False)
                nc.tensor.matmul(dra, gws[:, 4 * HOP:5 * HOP], rhs_r,
                                 start=False, stop=stop)
                nc.tensor.matmul(drb, gws[:, 5 * HOP:6 * HOP], rhs_r,
                                 start=False, stop=stop)

            # ---- software-pipelined emission ----
            ri_neg(0, 'v')
            r_roll(1)
            r_roll(2)
            r_roll(3)
            r_roll(4)
            c_group(0, 1, 'v')
            u_mm(0)
            u_evict(0)
            x0_tile()
            ri_neg(1, 'v')
            ri_neg(2, 'v')
            c_group(1, 2, 'v')
            u_mm(1)
            u_evict(1)
            ri_neg(3, 'v')
            ri_neg(4, 'v')
            c_group(3, 2, 'v')
            u_mm(2)
            u_evict(2)
            # V waves per j; incremental sums on gpsimd
            def vq_ap(k):
                return _ap(VQ[:, :], k * CD, [[4 * CD, 2], [1, CD]])

            def vall_ap(j):
                return _ap(Vall[:, :], j * CD, [[WD, 2], [1, CD]])

            v_wave(0, 1)
            u_mm(3)
            u_evict(3)
            v_wave(1, 1)
            TTv(vq_ap(0), vall_ap(0), vall_ap(1), ADD)
            u_mm(4)
            v_wave(2, 1)
            TTv(vq_ap(1), vq_ap(0), vall_ap(2), ADD)
            v_wave(3, 1)
            TTv(vq_ap(2), vq_ap(1), vall_ap(3), ADD)
            v_wave(4, 1, psum=True)
            TTv(vq_ap(3), vq_ap(2), vall_ap(4), ADD)
            d_pass(vq_ap(3),
                   _ap(VQ[:, :], 7 * CD, [[1, CD]]),
                   _ap(VQ[:, :], 3 * CD, [[1, CD]]), True)

            # ---- tail: evict Y (cov/2 folded in G), rescale edge cols ----
            Ysb = wpool.tile([HOP, 2 * BM * TP], BF16, tag="Ysb")
            nc.scalar.activation(Ysb[:, :], Y[:, :], CPY)
            nc.scalar.activation(
                _ap(Ysb[:, :], 0, [[CW, 2], [TP, BM], [T, 2]]),
                _ap(Y[:, :], 0, [[CW, 2], [TP, BM], [T, 2]]), CPY, scale=2.0)
            nc.sync.dma_start(yv[:, 0:CW], Ysb[:, 0:CW])
            nc.gpsimd.dma_start(yv[:, CW:2 * CW], Ysb[:, CW:2 * CW])
    return nc


# ---------------- host side ----------------

def _dft_consts():
    j = np.arange(F)
    W = np.exp(-2j * np.pi * np.outer(j, j) / F)
    G = np.exp(+2j * np.pi * np.outer(j, j) / F) / F
    return W, G


def _frame(sig):
    idx = np.arange(T)[None, :] * HOP + np.arange(F)[:, None]   # [g, t]
    return sig[idx].astype(np.float32)


def _m_mats(w2, n2):
    g = np.arange(F)[:, None]
    f = np.arange(F)[None, :]
    n1 = ((f - g + 20) % F) - 20
    valid = (n1 >= -20) & (n1 <= 19)
    n1c = np.clip(n1 + 20, 0, 39)
    col = w2[:, n2 + 20]
    Mr = np.where(valid, col.real[n1c], 0.0).astype(np.float32)
    Mi = np.where(valid, col.imag[n1c], 0.0).astype(np.float32)
    return Mr, Mi


def make_in_maps(x_real, x_imag, task_info, w_real, w_imag):
    W, G = _dft_consts()
    b, _, m = x_real.shape
    P = np.power(10.0, task_info[:, 0] / 10.0) / m
    w2 = (np.asarray(w_real) + 1j * np.asarray(w_imag)).reshape(40, 40)

    frs, fis = [], []
    for bb in range(b):
        s = float(P[bb]) ** (1.0 / 3.0)
        for mm in range(m):
            frs.append(_frame(x_real[bb, :, mm]) * s)
            fis.append(_frame(x_imag[bb, :, mm]) * s)
    fr = np.stack(frs, 1)
    fi = np.stack(fis, 1)
    xfv = np.concatenate([(-fi).reshape(F, -1), fr.reshape(F, -1),
                          fi.reshape(F, -1)], axis=1).astype(bfloat16)

    # G folded: 1/cov=1/2, rows split [0:40)/[40:80) for fused overlap-add
    Gh = G * 0.5
    gwv = np.concatenate([Gh.real[0:HOP].T, Gh.real[HOP:F].T,
                          -Gh.imag[0:HOP].T, -Gh.imag[HOP:F].T,
                          Gh.imag[0:HOP].T, Gh.imag[HOP:F].T],
                         axis=1).astype(bfloat16)

    # permutation matrices for rolls r=1..4 (lhsT[g, f] = 1 iff g=(f-r)%80)
    pparts = []
    g = np.arange(F)
    for r in range(1, NJ):
        Pm = np.zeros((F, F), np.float32)
        Pm[(g - r) % F, g] = 1.0
        pparts.append(Pm)
    pwv = np.concatenate(pparts, axis=1).astype(bfloat16)

    in_maps, shards = [], []
    for ci in range(8):
        sc = 5 * ci - 20
        Ws = np.roll(W, sc, axis=0).T
        fwv = np.concatenate([W.real, W.imag, Ws.real, Ws.imag],
                             axis=1).astype(bfloat16)
        mparts = []
        for r in range(NJ):
            Mr, Mi = _m_mats(w2, sc + r)
            mparts += [Mr, -Mi, Mi]
        mwv = np.concatenate(mparts, axis=1).astype(bfloat16)
        in_maps.append({"xf": xfv, "fw": fwv, "pw": pwv, "mw": mwv,
                        "gw": gwv})
        shards.append(ci)

    cov = np.zeros(L)
    idx = (np.arange(T)[:, None] * HOP + np.arange(F)[None, :]).reshape(-1)
    np.add.at(cov, idx, 1.0)
    cov = np.where(cov > 0, cov, 1.0)
    return in_maps, shards, P, cov


_NC_CACHE = {}


def kernel(x_real, x_imag, task_info, w_real, w_imag, b_real, b_imag):
    x_real = np.asarray(x_real)
    x_imag = np.asarray(x_imag)
    task_info = np.asarray(task_info)
    b, Lx, m = x_real.shape
    assert (b, Lx, m) == (2, L, 2)

    if "nc" not in _NC_CACHE:
        nc_ = build_program(debug=False)
        nc_.compile()
        _NC_CACHE["nc"] = nc_
    nc = _NC_CACHE["nc"]

    in_maps, shards, P, cov = make_in_maps(x_real, x_imag, task_info,
                                           w_real, w_imag)
    from concourse.bass_utils import run_bass_kernel_spmd
    res = run_bass_kernel_spmd(nc, in_maps, list(range(8))).results

    CW = BM * TP
    Ysum = np.zeros((HOP, 2 * CW), np.float64)
    for i in range(8):
        Ysum += np.asarray(res[i]["yv"], np.float64)
    Y = Ysum.reshape(HOP, 2, BM, TP)

    x = (x_real + 1j * x_imag).astype(np.complex64)
    out = x.copy()
    bias = complex(np.asarray(b_real)[0], np.asarray(b_imag)[0])
    bias_sig = np.zeros(L, np.complex64)
    bias_sig[np.arange(T) * HOP] = bias
    bias_sig /= cov
    for u in range(BM):
        bb, mm = divmod(u, m)
        yr = Y[:, 0, u].T.ravel()[:L]
        yi = Y[:, 1, u].T.ravel()[:L]
        out[bb, :, mm] += (yr + 1j * yi).astype(np.complex64)
        out[bb, :, mm] += (P[bb] * bias_sig).astype(np.complex64)
    return out[:, 20:L - 20, :]
